# revision 26
# baseline (speedup 1.0000x reference)
"""Causal self-attention Bass kernel for 8x Trainium2 NeuronCores.

Problem: B=8, T=1024, D=1024, H=16 heads (head_dim 64), fp32.
Sharding: data parallel over batch -- each of the 8 cores handles one
batch element with replicated weights; outputs are stacked on the host.

v2 design (vs the 339us baseline): the baseline ran the whole attention
phase with the PE at K=4/8 (HAM-cold, half clock) and serialized 156us
of ACT work after the GEMM phase.  This version:
  * transposes + bf16-casts x on the host (kills 64 PE transposes and
    their DVE evacuations; weights were already host-prepped in the
    baseline),
  * runs the per-head-pair attention chains *interleaved* with the next
    pair's qkv GEMM slice, so the PE always has dense independent matmul
    work while ACT exp catches up (stays HAM-warm),
  * pairs the two heads of an f-tile in concurrent row-tiled QK matmuls
    (tile_position (0,0)/(64,0), K=64 each) -- 2x QK throughput,
  * one exp ACT call per (i-tile, head-pair) over [128,2,512-ws], and
    softmax 1/denom via batched Ln/Exp on [2,512] rows per (pair, j)
    instead of 64 separate [1,512] calls,
  * exact causal windows (width 512-128*(i-4j)).

Per-core dataflow (all matmuls on PE in bf16 with fp32 PSUM accumulate):
  1. v = x @ w_qkv[:, 2048:] + bias (ones-row matmul), stored per
     tk-tile with an interleaved ones column (denominator trick).
  2. Per head pair t: qT/kT f-tiles [128, T] from w_qkv.T @ x.T with
     bias folded into the PSUM->SBUF evacuation (per-partition add).
  3. Per (pair, j-block, i-tile): sT[tk,tq] = kT.T @ qT for both heads
     concurrently (row-tiled), exp on ACT (scale 1/8), triangular mask
     multiply on diagonal tiles, o_aug[65,tq] += v_aug.T @ P.
  4. 1/denom = exp(-ln(d)) batched on [2,512], gpsimd broadcast,
     normalize fused into the PSUM evacuation (DVE tensor_tensor).
  5. y = attT.T @ w_proj + b_proj (ones-row matmul), streamed to DRAM.
"""

import numpy as np
from contextlib import ExitStack

import concourse.bass as bass
import concourse.bacc as bacc
import concourse.tile as tile
import concourse.mybir as mybir
from concourse import bass_utils

F32 = mybir.dt.float32
BF16 = mybir.dt.bfloat16
AF = mybir.ActivationFunctionType
OP = mybir.AluOpType

B, T, D, H, HD = 8, 1024, 1024, 16, 64
P = 128
N_CORES = 8

TRACE = False
_CACHE = {}
LAST_RESULT = {}


def _build_tile_kernel(nc, aps):
    xt, wqk, vw, wp, bcol, bv, bp, tri, ones, out = (
        aps["xt"], aps["wqk"], aps["vw"], aps["wp"], aps["bcol"],
        aps["bv"], aps["bp"], aps["tri"], aps["ones"], aps["out"],
    )

    with tile.TileContext(nc) as tc, ExitStack() as ctx:
        consts = ctx.enter_context(tc.tile_pool(name="consts", bufs=1))
        big = ctx.enter_context(tc.tile_pool(name="big", bufs=1))
        # all 16 qT/kT f-tiles stay live across both j sweeps
        qk_pool = ctx.enter_context(tc.tile_pool(name="qk_pool", bufs=16))
        v_pool = ctx.enter_context(tc.tile_pool(name="v_pool", bufs=8))
        at_pool = ctx.enter_context(tc.tile_pool(name="at_pool", bufs=16))
        p_pool = ctx.enter_context(tc.tile_pool(name="p_pool", bufs=6))
        nrm_pool = ctx.enter_context(tc.tile_pool(name="nrm_pool", bufs=4))
        row_pool = ctx.enter_context(tc.tile_pool(name="row_pool", bufs=4))
        y_pool = ctx.enter_context(tc.tile_pool(name="y_pool", bufs=3))
        # PSUM budget (8 banks): s2 scores 2x[128,1024] = 4, GEMM/proj
        # accumulator halves 2x[128,512] = 2, o_aug 2x[65,512] = 2.
        ps2 = ctx.enter_context(tc.tile_pool(name="ps2", bufs=2, space="PSUM"))
        psg = ctx.enter_context(tc.tile_pool(name="psg", bufs=2, space="PSUM"))
        ops = ctx.enter_context(tc.tile_pool(name="ops", bufs=2, space="PSUM"))

        # ---- constants -------------------------------------------------
        tri_sb = consts.tile([P, P], BF16)
        nc.sync.dma_start(out=tri_sb, in_=tri)
        ones_sb = consts.tile([1, P], BF16)
        nc.sync.dma_start(out=ones_sb, in_=ones)
        bcol_sb = consts.tile([P, 16], F32)  # b_qkv[0:2048] as per-partition cols
        nc.sync.dma_start(out=bcol_sb, in_=bcol)
        bv_sb = consts.tile([1, D], BF16)  # v bias as a row
        nc.sync.dma_start(out=bv_sb, in_=bv)
        bp_sb = consts.tile([1, D], BF16)
        nc.sync.dma_start(out=bp_sb, in_=bp)

        # ---- big input tiles (one tile per k-slice so readers only wait
        # for their own slice's DMA), spread across three engine queues in
        # need-order: pair-0 qk weights + x^T first, then v weights, later
        # pairs, w_proj last
        xt_sb = [big.tile([P, T], BF16, name=f"xt{k}") for k in range(8)]
        vw_sb = [big.tile([P, T], BF16, name=f"vw{k}") for k in range(8)]
        wqk_sb = [big.tile([P, 2048], BF16, name=f"wqk{t}") for t in range(8)]
        wp_sb = [big.tile([P, T], BF16, name=f"wp{c}") for c in range(8)]
        qs = [nc.sync, nc.gpsimd, nc.scalar]
        nc.scalar.dma_start(out=wqk_sb[0], in_=wqk[0])
        for k in range(8):
            qs[k % 3].dma_start(out=xt_sb[k], in_=xt[k])
        for k in range(8):
            (nc.sync if k % 2 == 0 else nc.gpsimd).dma_start(
                out=vw_sb[k], in_=vw[k])
        for t in range(1, 8):
            qs[t % 3].dma_start(out=wqk_sb[t], in_=wqk[t])
        for c in range(8):
            (nc.scalar if c % 2 == 0 else nc.sync).dma_start(
                out=wp_sb[c], in_=wp[c])

        # ---- phase V: v in natural layout with interleaved ones col ----
        v_tiles = []
        # per-head stride 66 (132B) keeps every head's va slice 4B-aligned
        for m in range(8):
            vt = v_pool.tile([P, 16 * 66], BF16, name="vt", tag="vt")
            nc.vector.memset(
                vt.rearrange("p (h c) -> p h c", c=66)[:, :, 64:66], 1.0
            )
            v_tiles.append(vt)
        # ---- qk GEMM for one pair: qT then kT f-tiles ------------------
        qk_tiles = {}  # t -> (q_tile, k_tile), each [128, T] bf16

        def emit_qk_gemm(t):
            tiles = []
            for which in (0, 1):  # 0 = q f-tile, 1 = k f-tile
                f = t if which == 0 else 8 + t
                qk_t = qk_pool.tile([P, T], BF16, name="qk_t", tag="qk")
                for n in range(2):
                    acc = psg.tile([P, 512], F32, name="qkacc", tag="psg")
                    for k in range(8):
                        wsl = wqk_sb[t][:, which * 1024 + k * P:
                                        which * 1024 + (k + 1) * P]
                        nc.tensor.matmul(acc, wsl,
                                         xt_sb[k][:, n * 512:(n + 1) * 512],
                                         start=(k == 0), stop=(k == 7))
                    # evac on ACT (Identity + per-partition bias) -- keeps
                    # the DVE stream free of GEMM-gated blocking
                    nc.scalar.activation(qk_t[:, n * 512:(n + 1) * 512], acc,
                                         AF.Identity, bias=bcol_sb[:, f:f + 1])
                tiles.append(qk_t)
            qk_tiles[t] = tuple(tiles)

        emit_qk_gemm(0)
        emit_qk_gemm(1)

        # v-GEMM after the first two qk GEMMs: the qk path needs only
        # wqk[0] + x^T (2.5MB) so the PE starts ~15us earlier
        for m in range(8):
            rr = v_tiles[m].rearrange("p (h c) -> p h c", c=66)
            for n in range(2):
                acc = psg.tile([P, 512], F32, name="vacc", tag="psg")
                for k in range(8):
                    xsl = xt_sb[k][:, m * P:(m + 1) * P]
                    nc.tensor.matmul(acc, xsl, vw_sb[k][:, n * 512:(n + 1) * 512],
                                     start=(k == 0), stop=False)
                nc.tensor.matmul(acc, ones_sb, bv_sb[:, n * 512:(n + 1) * 512],
                                 start=False, stop=True)
                nc.vector.tensor_copy(rr[:, 8 * n:8 * n + 8, 0:64], acc)


        # ---- attention chains, j-outer ---------------------------------
        # denom staging ring: rows 0 and 32 hold the two heads' denoms
        # (32-aligned partition bases); other rows memset once so the
        # batched Ln never reads uninitialized SBUF.
        dstages, rrows, rrbs = [], [], []
        for _ in range(4):
            ds = row_pool.tile([33, 512], F32, name="dstage", tag="dst")
            nc.vector.memset(ds, 1.0)
            dstages.append(ds)
            rrows.append(row_pool.tile([33, 512], F32, name="rrow", tag="rr"))
            rrbs.append(row_pool.tile([33, 512], BF16, name="rrb", tag="rrb"))

        att_tiles = {}  # (t, j) -> [128, 512] bf16 (head 2t rows 0:64, 2t+1 64:128)
        pending = []    # deferred softmax normalizations

        def flush_norm():
            """Emit the deferred 1/denom + scale for the previous block.
            Deferring keeps the norm's ACT ops out of the exp stream's way
            (no head-of-line blocking) and off the o_ps critical path."""
            if not pending:
                return
            at, dstage, rrow, rrb = pending.pop(0)
            nc.scalar.activation(rrow, dstage, AF.Ln)
            nc.scalar.activation(rrb, rrow, AF.Exp, scale=-1.0)
            rtmp = row_pool.tile([1, 512], BF16, name="rtmp", tag="rt")
            nc.vector.tensor_copy(rtmp, rrb[32:33, :])
            # one [128,512] recip tile: head A rows 0:64, head B rows 64:128,
            # so the scale is a single full-width base-0 DVE multiply
            rb = nrm_pool.tile([P, 512], BF16, name="rb", tag="rb")
            nc.gpsimd.partition_broadcast(rb[0:64, :], rrb[0:1, :])
            # gpsimd can't write at partition base 64: bounce via base-0 tile
            rbB = nrm_pool.tile([64, 512], BF16, name="rbB", tag="rbB")
            nc.gpsimd.partition_broadcast(rbB, rtmp)
            nc.vector.tensor_copy(rb[64:128, :], rbB)
            nc.vector.tensor_tensor(at, at, rb, op=OP.mult)

        def emit_chain(t, j):
            q_t, k_t = qk_tiles[t]
            at = at_pool.tile([P, 512], BF16, name="at", tag="at")
            att_tiles[(t, j)] = at
            o_A = ops.tile([65, 512], F32, name="o_A", tag="ops")
            o_B = ops.tile([65, 512], F32, name="o_B", tag="ops")
            ni = 4 * j + 4
            for i in range(ni):
                ws = max(0, P * (i - 4 * j))
                s2 = ps2.tile([P, 2, 512], F32, name="s2", tag="ps2")
                p2 = p_pool.tile([P, 2, 512], BF16, name="p2", tag="p2")
                for hh in range(2):
                    po = hh * 64
                    kT = k_t[po:po + 64, i * P:(i + 1) * P]
                    qT = q_t[po:po + 64, j * 512 + ws:(j + 1) * 512]
                    nc.tensor.matmul(s2[:, hh, ws:], kT, qT,
                                     start=True, stop=True)
                nc.scalar.activation(p2[:, :, ws:], s2[:, :, ws:],
                                     AF.Exp, scale=0.125)
                if i >= 4 * j:
                    # diagonal block: zero the strict upper triangle
                    for hh in range(2):
                        nc.vector.tensor_tensor(
                            p2[:, hh, ws:ws + P], p2[:, hh, ws:ws + P],
                            tri_sb, op=OP.mult,
                        )
                for hh, o_ps in ((0, o_A), (1, o_B)):
                    h = 2 * t + hh
                    va = v_tiles[i].rearrange(
                        "p (h c) -> p h c", c=66)[:, h, 0:65]
                    nc.tensor.matmul(o_ps[:, ws:], va, p2[:, hh, ws:],
                                     start=(i == 0), stop=(i == ni - 1))
            # stage denominators + evacuate unnormalized o (frees o banks
            # quickly); the recip + scale runs deferred via flush_norm()
            dstage = dstages[(2 * t + j) % 4]
            rrow = rrows[(2 * t + j) % 4]
            rrb = rrbs[(2 * t + j) % 4]
            nc.vector.tensor_copy(dstage[0:1, :], o_A[64:65, :])
            nc.vector.tensor_copy(dstage[32:33, :], o_B[64:65, :])
            nc.vector.tensor_copy(at[0:64, :], o_A[0:64, :])
            nc.vector.tensor_copy(at[64:128, :], o_B[0:64, :])
            pending.append((at, dstage, rrow, rrb))

        def emit_proj(mrow):
            j, mi = mrow // 4, mrow % 4
            y_sb = y_pool.tile([P, T], F32, name="y_sb", tag="y")
            for n in range(2):
                y_ps = psg.tile([P, 512], F32, name="y_ps", tag="psg")
                for c in range(8):
                    asl = att_tiles[(c, j)][:, mi * P:(mi + 1) * P]
                    nc.tensor.matmul(y_ps, asl,
                                     wp_sb[c][:, n * 512:(n + 1) * 512],
                                     start=(c == 0), stop=False)
                nc.tensor.matmul(y_ps, ones_sb, bp_sb[:, n * 512:(n + 1) * 512],
                                 start=False, stop=True)
                nc.vector.tensor_copy(y_sb[:, n * 512:(n + 1) * 512], y_ps)
            nc.sync.dma_start(out=out[mrow * P:(mrow + 1) * P, :], in_=y_sb)

        # sweep j=0 with the qk GEMMs as PE filler (filler first so its
        # PSUM evacuations overlap the chain instead of gating the next one)
        for t in range(8):
            if t < 6:
                emit_qk_gemm(t + 2)
            emit_chain(t, 0)
            flush_norm()
        # sweep j=1 with the j=0 projection rows as PE filler
        for t in range(8):
            if t % 2 == 1:
                emit_proj(t // 2)
            emit_chain(t, 1)
            flush_norm()
        # remaining projection rows (j=1)
        flush_norm()
        for mrow in range(4, 8):
            emit_proj(mrow)


def _pin_act_table(arch):
    """Force every ACT func we use into one table so walrus never emits
    mid-kernel ACT_TABLE_LOADs (each is ~1.3us on the ScalarE stream)."""
    import concourse.hw_specs as hw_specs
    tabs = hw_specs.get_activation_tables(arch)
    keep = "natural_log_exp_and_others"
    if keep not in tabs:
        return
    need = tabs[keep] & {AF.Exp, AF.Ln, AF.Copy, AF.Identity}
    for name, fns in tabs.items():
        if name != keep:
            fns -= need


def _get_nc():
    if "nc" in _CACHE:
        return _CACHE["nc"]
    nc = bacc.Bacc("TRN2", target_bir_lowering=False, debug=False,
                   num_devices=N_CORES)
    _pin_act_table(nc.m.arch)
    aps = {
        "xt": nc.dram_tensor("xt", [8, P, T], BF16, kind="ExternalInput").ap(),
        "wqk": nc.dram_tensor("wqk", [8, P, 2048], BF16, kind="ExternalInput").ap(),
        "vw": nc.dram_tensor("vw", [8, P, T], BF16, kind="ExternalInput").ap(),
        "wp": nc.dram_tensor("wp", [8, P, T], BF16, kind="ExternalInput").ap(),
        "bcol": nc.dram_tensor("bcol", [P, 16], F32, kind="ExternalInput").ap(),
        "bv": nc.dram_tensor("bv", [1, D], BF16, kind="ExternalInput").ap(),
        "bp": nc.dram_tensor("bp", [1, D], BF16, kind="ExternalInput").ap(),
        "tri": nc.dram_tensor("tri", [P, P], BF16, kind="ExternalInput").ap(),
        "ones": nc.dram_tensor("ones", [1, P], BF16, kind="ExternalInput").ap(),
        "out": nc.dram_tensor("out", [T, D], F32, kind="ExternalOutput").ap(),
    }
    _build_tile_kernel(nc, aps)
    nc.compile()
    _CACHE["nc"] = nc
    return nc


def kernel(x, w_qkv, b_qkv, w_proj, b_proj):
    import ml_dtypes
    bf = ml_dtypes.bfloat16

    x = np.ascontiguousarray(np.asarray(x, dtype=np.float32))
    w_qkv = np.asarray(w_qkv, dtype=np.float32)
    b_qkv = np.asarray(b_qkv, dtype=np.float32)
    w_proj = np.asarray(w_proj, dtype=np.float32)
    b_proj = np.asarray(b_proj, dtype=np.float32)

    nc = _get_nc()

    # host-side input prep (dtype cast + layout), shared across cores
    wq = w_qkv[:, :2048].astype(bf)                      # [D, 2048]
    # per pair t: q f-tile t (cols 128t..) then k f-tile t (cols 1024+128t..),
    # each as [128(k-part), 8(k-tile), 128(f)] flattened to [128, 1024]
    wq4 = wq.reshape(8, P, 16, P)                        # [k, p, f, m]
    wqk_prep = np.empty((8, P, 2048), dtype=bf)
    for t in range(8):
        wqk_prep[t, :, 0:1024] = (
            wq4[:, :, t, :].transpose(1, 0, 2).reshape(P, 1024))
        wqk_prep[t, :, 1024:2048] = (
            wq4[:, :, 8 + t, :].transpose(1, 0, 2).reshape(P, 1024))
    vw_prep = np.ascontiguousarray(
        w_qkv[:, 2048:].astype(bf).reshape(8, P, T))     # [k, p, n]
    wp_prep = np.ascontiguousarray(
        w_proj.astype(bf).reshape(8, P, T))              # [c, p, n]
    bcol = np.ascontiguousarray(
        b_qkv[0:2048].reshape(16, P).T.astype(np.float32))
    bv = b_qkv[2048:3072].reshape(1, D).astype(bf)
    bp = b_proj.reshape(1, D).astype(bf)
    r = np.arange(P)
    tri = (r[:, None] <= r[None, :]).astype(bf)
    ones = np.ones((1, P), dtype=bf)

    shared = {
        "wqk": wqk_prep, "vw": vw_prep, "wp": wp_prep,
        "bcol": bcol, "bv": bv, "bp": bp, "tri": tri, "ones": ones,
    }
    in_maps = []
    for b in range(N_CORES):
        xtb = np.ascontiguousarray(
            x[b].T.astype(bf).reshape(8, P, T))          # [k, p, t]
        in_maps.append(dict(shared, xt=xtb))

    res = bass_utils.run_bass_kernel_spmd(
        nc, in_maps, core_ids=list(range(N_CORES)), trace=TRACE
    )
    LAST_RESULT["res"] = res
    return np.stack([res.results[c]["out"] for c in range(N_CORES)]).astype(
        np.float32
    )


# revision 27
# speedup vs baseline: 1.0316x; 1.0316x over previous
"""Causal self-attention Bass kernel for 8x Trainium2 NeuronCores.

Problem: B=8, T=1024, D=1024, H=16 heads (head_dim 64), fp32.
Sharding: data parallel over batch -- each of the 8 cores handles one
batch element with replicated weights; outputs are stacked on the host.

v2 design (vs the 339us baseline): the baseline ran the whole attention
phase with the PE at K=4/8 (HAM-cold, half clock) and serialized 156us
of ACT work after the GEMM phase.  This version:
  * transposes + bf16-casts x on the host (kills 64 PE transposes and
    their DVE evacuations; weights were already host-prepped in the
    baseline),
  * runs the per-head-pair attention chains *interleaved* with the next
    pair's qkv GEMM slice, so the PE always has dense independent matmul
    work while ACT exp catches up (stays HAM-warm),
  * pairs the two heads of an f-tile in concurrent row-tiled QK matmuls
    (tile_position (0,0)/(64,0), K=64 each) -- 2x QK throughput,
  * one exp ACT call per (i-tile, head-pair) over [128,2,512-ws], and
    softmax 1/denom via batched Ln/Exp on [2,512] rows per (pair, j)
    instead of 64 separate [1,512] calls,
  * exact causal windows (width 512-128*(i-4j)).

Per-core dataflow (all matmuls on PE in bf16 with fp32 PSUM accumulate):
  1. v = x @ w_qkv[:, 2048:] + bias (ones-row matmul), stored per
     tk-tile with an interleaved ones column (denominator trick).
  2. Per head pair t: qT/kT f-tiles [128, T] from w_qkv.T @ x.T with
     bias folded into the PSUM->SBUF evacuation (per-partition add).
  3. Per (pair, j-block, i-tile): sT[tk,tq] = kT.T @ qT for both heads
     concurrently (row-tiled), exp on ACT (scale 1/8), triangular mask
     multiply on diagonal tiles, o_aug[65,tq] += v_aug.T @ P.
  4. 1/denom = exp(-ln(d)) batched on [2,512], gpsimd broadcast,
     normalize fused into the PSUM evacuation (DVE tensor_tensor).
  5. y = attT.T @ w_proj + b_proj (ones-row matmul), streamed to DRAM.
"""

import numpy as np
from contextlib import ExitStack

import concourse.bass as bass
import concourse.bacc as bacc
import concourse.tile as tile
import concourse.mybir as mybir
from concourse import bass_utils

F32 = mybir.dt.float32
BF16 = mybir.dt.bfloat16
AF = mybir.ActivationFunctionType
OP = mybir.AluOpType

B, T, D, H, HD = 8, 1024, 1024, 16, 64
P = 128
N_CORES = 8

TRACE = False
_CACHE = {}
LAST_RESULT = {}


def _build_tile_kernel(nc, aps):
    xt, wqk, vw, wp, bcol, bv, bp, tri, ones, out = (
        aps["xt"], aps["wqk"], aps["vw"], aps["wp"], aps["bcol"],
        aps["bv"], aps["bp"], aps["tri"], aps["ones"], aps["out"],
    )

    with tile.TileContext(nc) as tc, ExitStack() as ctx:
        consts = ctx.enter_context(tc.tile_pool(name="consts", bufs=1))
        big = ctx.enter_context(tc.tile_pool(name="big", bufs=1))
        # all 16 qT/kT f-tiles stay live across both j sweeps
        qk_pool = ctx.enter_context(tc.tile_pool(name="qk_pool", bufs=16))
        v_pool = ctx.enter_context(tc.tile_pool(name="v_pool", bufs=8))
        at_pool = ctx.enter_context(tc.tile_pool(name="at_pool", bufs=16))
        p_pool = ctx.enter_context(tc.tile_pool(name="p_pool", bufs=6))
        nrm_pool = ctx.enter_context(tc.tile_pool(name="nrm_pool", bufs=4))
        row_pool = ctx.enter_context(tc.tile_pool(name="row_pool", bufs=4))
        y_pool = ctx.enter_context(tc.tile_pool(name="y_pool", bufs=3))
        # PSUM: ps2 tiles are [128,1024] (2 banks); 3 bufs = 6 banks.
        # o_ps tiles are [65,512] (1 bank); 2 bufs.  Total 8 banks.
        ps2 = ctx.enter_context(tc.tile_pool(name="ps2", bufs=3, space="PSUM"))
        ops = ctx.enter_context(tc.tile_pool(name="ops", bufs=2, space="PSUM"))

        # ---- constants -------------------------------------------------
        tri_sb = consts.tile([P, P], BF16)
        nc.sync.dma_start(out=tri_sb, in_=tri)
        ones_sb = consts.tile([1, P], BF16)
        nc.sync.dma_start(out=ones_sb, in_=ones)
        bcol_sb = consts.tile([P, 16], F32)  # b_qkv[0:2048] as per-partition cols
        nc.sync.dma_start(out=bcol_sb, in_=bcol)
        bv_sb = consts.tile([1, D], BF16)  # v bias as a row
        nc.sync.dma_start(out=bv_sb, in_=bv)
        bp_sb = consts.tile([1, D], BF16)
        nc.sync.dma_start(out=bp_sb, in_=bp)

        # ---- big input tiles (one tile per k-slice so readers only wait
        # for their own slice's DMA), spread across three engine queues in
        # need-order: pair-0 qk weights + x^T first, then v weights, later
        # pairs, w_proj last
        xt_sb = [big.tile([P, T], BF16, name=f"xt{k}") for k in range(8)]
        vw_sb = [big.tile([P, T], BF16, name=f"vw{k}") for k in range(8)]
        wqk_sb = [big.tile([P, 2048], BF16, name=f"wqk{t}") for t in range(8)]
        wp_sb = [big.tile([P, T], BF16, name=f"wp{c}") for c in range(8)]
        qs = [nc.sync, nc.gpsimd, nc.scalar]
        nc.scalar.dma_start(out=wqk_sb[0], in_=wqk[0])
        for k in range(8):
            qs[k % 3].dma_start(out=xt_sb[k], in_=xt[k])
        for k in range(8):
            (nc.sync if k % 2 == 0 else nc.gpsimd).dma_start(
                out=vw_sb[k], in_=vw[k])
        for t in range(1, 8):
            qs[t % 3].dma_start(out=wqk_sb[t], in_=wqk[t])
        for c in range(8):
            (nc.scalar if c % 2 == 0 else nc.sync).dma_start(
                out=wp_sb[c], in_=wp[c])

        # ---- phase V: v in natural layout with interleaved ones col ----
        v_tiles = []
        # per-head stride 66 (132B) keeps every head's va slice 4B-aligned
        for m in range(8):
            vt = v_pool.tile([P, 16 * 66], BF16, name="vt", tag="vt")
            nc.vector.memset(
                vt.rearrange("p (h c) -> p h c", c=66)[:, :, 64:66], 1.0
            )
            v_tiles.append(vt)
        # ---- qk GEMM for one pair: qT then kT f-tiles ------------------
        qk_tiles = {}  # t -> (q_tile, k_tile), each [128, T] bf16

        def emit_qk_gemm(t):
            tiles = []
            for which in (0, 1):  # 0 = q f-tile, 1 = k f-tile
                f = t if which == 0 else 8 + t
                qk_t = qk_pool.tile([P, T], BF16, name="qk_t", tag="qk")
                acc = ps2.tile([P, T], F32, name="qkacc", tag="ps2")
                for k in range(8):
                    wsl = wqk_sb[t][:, which * 1024 + k * P:
                                    which * 1024 + (k + 1) * P]
                    nc.tensor.matmul(acc[:, 0:512], wsl, xt_sb[k][:, 0:512],
                                     start=(k == 0), stop=(k == 7))
                    nc.tensor.matmul(acc[:, 512:1024], wsl,
                                     xt_sb[k][:, 512:1024],
                                     start=(k == 0), stop=(k == 7))
                # evac on ACT (Identity + per-partition bias) -- keeps the
                # DVE stream free of GEMM-gated head-of-line blocking
                nc.scalar.activation(qk_t[:, 0:512], acc[:, 0:512],
                                     AF.Identity, bias=bcol_sb[:, f:f + 1])
                nc.scalar.activation(qk_t[:, 512:1024], acc[:, 512:1024],
                                     AF.Identity, bias=bcol_sb[:, f:f + 1])
                tiles.append(qk_t)
            qk_tiles[t] = tuple(tiles)

        emit_qk_gemm(0)
        emit_qk_gemm(1)

        # v-GEMM after the first two qk GEMMs: the qk path needs only
        # wqk[0] + x^T (2.5MB) so the PE starts ~15us earlier
        for m in range(8):
            acc = ps2.tile([P, T], F32, name="vacc", tag="ps2")
            for k in range(8):
                xsl = xt_sb[k][:, m * P:(m + 1) * P]
                nc.tensor.matmul(acc[:, 0:512], xsl, vw_sb[k][:, 0:512],
                                 start=(k == 0), stop=False)
                nc.tensor.matmul(acc[:, 512:1024], xsl, vw_sb[k][:, 512:1024],
                                 start=(k == 0), stop=False)
            nc.tensor.matmul(acc[:, 0:512], ones_sb, bv_sb[:, 0:512],
                             start=False, stop=True)
            nc.tensor.matmul(acc[:, 512:1024], ones_sb, bv_sb[:, 512:1024],
                             start=False, stop=True)
            rr = v_tiles[m].rearrange("p (h c) -> p h c", c=66)
            nc.vector.tensor_copy(rr[:, 0:8, 0:64], acc[:, 0:512])
            nc.vector.tensor_copy(rr[:, 8:16, 0:64], acc[:, 512:1024])


        # ---- attention chains, j-outer ---------------------------------
        # denom staging ring: rows 0 and 32 hold the two heads' denoms
        # (32-aligned partition bases); other rows memset once so the
        # batched Ln never reads uninitialized SBUF.
        dstages, rrows, rrbs = [], [], []
        for _ in range(4):
            ds = row_pool.tile([33, 512], F32, name="dstage", tag="dst")
            nc.vector.memset(ds, 1.0)
            dstages.append(ds)
            rrows.append(row_pool.tile([33, 512], F32, name="rrow", tag="rr"))
            rrbs.append(row_pool.tile([33, 512], BF16, name="rrb", tag="rrb"))

        att_tiles = {}  # (t, j) -> [128, 512] bf16 (head 2t rows 0:64, 2t+1 64:128)
        pending = []    # deferred softmax normalizations

        def flush_norm():
            """Emit the deferred 1/denom + scale for the previous block.
            Deferring keeps the norm's ACT ops out of the exp stream's way
            (no head-of-line blocking) and off the o_ps critical path."""
            if not pending:
                return
            at, dstage, rrow, rrb = pending.pop(0)
            nc.scalar.activation(rrow, dstage, AF.Ln)
            nc.scalar.activation(rrb, rrow, AF.Exp, scale=-1.0)
            rtmp = row_pool.tile([1, 512], BF16, name="rtmp", tag="rt")
            nc.vector.tensor_copy(rtmp, rrb[32:33, :])
            # one [128,512] recip tile: head A rows 0:64, head B rows 64:128,
            # so the scale is a single full-width base-0 DVE multiply
            rb = nrm_pool.tile([P, 512], BF16, name="rb", tag="rb")
            nc.gpsimd.partition_broadcast(rb[0:64, :], rrb[0:1, :])
            # gpsimd can't write at partition base 64: bounce via base-0 tile
            rbB = nrm_pool.tile([64, 512], BF16, name="rbB", tag="rbB")
            nc.gpsimd.partition_broadcast(rbB, rtmp)
            nc.vector.tensor_copy(rb[64:128, :], rbB)
            nc.vector.tensor_tensor(at, at, rb, op=OP.mult)

        def emit_chain(t, j):
            q_t, k_t = qk_tiles[t]
            at = at_pool.tile([P, 512], BF16, name="at", tag="at")
            att_tiles[(t, j)] = at
            o_A = ops.tile([65, 512], F32, name="o_A", tag="ops")
            o_B = ops.tile([65, 512], F32, name="o_B", tag="ops")
            ni = 4 * j + 4
            for i in range(ni):
                ws = max(0, P * (i - 4 * j))
                s2 = ps2.tile([P, 2, 512], F32, name="s2", tag="ps2")
                p2 = p_pool.tile([P, 2, 512], BF16, name="p2", tag="p2")
                for hh in range(2):
                    po = hh * 64
                    kT = k_t[po:po + 64, i * P:(i + 1) * P]
                    qT = q_t[po:po + 64, j * 512 + ws:(j + 1) * 512]
                    nc.tensor.matmul(s2[:, hh, ws:], kT, qT,
                                     start=True, stop=True)
                nc.scalar.activation(p2[:, :, ws:], s2[:, :, ws:],
                                     AF.Exp, scale=0.125)
                if i >= 4 * j:
                    # diagonal block: zero the strict upper triangle
                    for hh in range(2):
                        nc.vector.tensor_tensor(
                            p2[:, hh, ws:ws + P], p2[:, hh, ws:ws + P],
                            tri_sb, op=OP.mult,
                        )
                for hh, o_ps in ((0, o_A), (1, o_B)):
                    h = 2 * t + hh
                    va = v_tiles[i].rearrange(
                        "p (h c) -> p h c", c=66)[:, h, 0:65]
                    nc.tensor.matmul(o_ps[:, ws:], va, p2[:, hh, ws:],
                                     start=(i == 0), stop=(i == ni - 1))
            # stage denominators + evacuate unnormalized o (frees o banks
            # quickly); the recip + scale runs deferred via flush_norm()
            dstage = dstages[(2 * t + j) % 4]
            rrow = rrows[(2 * t + j) % 4]
            rrb = rrbs[(2 * t + j) % 4]
            nc.vector.tensor_copy(dstage[0:1, :], o_A[64:65, :])
            nc.vector.tensor_copy(dstage[32:33, :], o_B[64:65, :])
            nc.vector.tensor_copy(at[0:64, :], o_A[0:64, :])
            nc.vector.tensor_copy(at[64:128, :], o_B[0:64, :])
            pending.append((at, dstage, rrow, rrb))

        def emit_proj(mrow):
            j, mi = mrow // 4, mrow % 4
            y_ps = ps2.tile([P, T], F32, name="y_ps", tag="ps2")
            for c in range(8):
                asl = att_tiles[(c, j)][:, mi * P:(mi + 1) * P]
                nc.tensor.matmul(y_ps[:, 0:512], asl, wp_sb[c][:, 0:512],
                                 start=(c == 0), stop=False)
                nc.tensor.matmul(y_ps[:, 512:1024], asl, wp_sb[c][:, 512:1024],
                                 start=(c == 0), stop=False)
            nc.tensor.matmul(y_ps[:, 0:512], ones_sb, bp_sb[:, 0:512],
                             start=False, stop=True)
            nc.tensor.matmul(y_ps[:, 512:1024], ones_sb, bp_sb[:, 512:1024],
                             start=False, stop=True)
            y_sb = y_pool.tile([P, T], F32, name="y_sb", tag="y")
            nc.vector.tensor_copy(y_sb, y_ps)
            nc.sync.dma_start(out=out[mrow * P:(mrow + 1) * P, :], in_=y_sb)

        # sweep j=0 with the qk GEMMs as PE filler (filler first so its
        # PSUM evacuations overlap the chain instead of gating the next one)
        for t in range(8):
            if t < 6:
                emit_qk_gemm(t + 2)
            emit_chain(t, 0)
            flush_norm()
        # sweep j=1 with the j=0 projection rows as PE filler
        for t in range(8):
            if t % 2 == 1:
                emit_proj(t // 2)
            emit_chain(t, 1)
            flush_norm()
        # remaining projection rows (j=1)
        flush_norm()
        for mrow in range(4, 8):
            emit_proj(mrow)


def _pin_act_table(arch):
    """Force every ACT func we use into one table so walrus never emits
    mid-kernel ACT_TABLE_LOADs (each is ~1.3us on the ScalarE stream)."""
    import concourse.hw_specs as hw_specs
    tabs = hw_specs.get_activation_tables(arch)
    keep = "natural_log_exp_and_others"
    if keep not in tabs:
        return
    need = tabs[keep] & {AF.Exp, AF.Ln, AF.Copy, AF.Identity}
    for name, fns in tabs.items():
        if name != keep:
            fns -= need


def _get_nc():
    if "nc" in _CACHE:
        return _CACHE["nc"]
    nc = bacc.Bacc("TRN2", target_bir_lowering=False, debug=False,
                   num_devices=N_CORES)
    _pin_act_table(nc.m.arch)
    aps = {
        "xt": nc.dram_tensor("xt", [8, P, T], BF16, kind="ExternalInput").ap(),
        "wqk": nc.dram_tensor("wqk", [8, P, 2048], BF16, kind="ExternalInput").ap(),
        "vw": nc.dram_tensor("vw", [8, P, T], BF16, kind="ExternalInput").ap(),
        "wp": nc.dram_tensor("wp", [8, P, T], BF16, kind="ExternalInput").ap(),
        "bcol": nc.dram_tensor("bcol", [P, 16], F32, kind="ExternalInput").ap(),
        "bv": nc.dram_tensor("bv", [1, D], BF16, kind="ExternalInput").ap(),
        "bp": nc.dram_tensor("bp", [1, D], BF16, kind="ExternalInput").ap(),
        "tri": nc.dram_tensor("tri", [P, P], BF16, kind="ExternalInput").ap(),
        "ones": nc.dram_tensor("ones", [1, P], BF16, kind="ExternalInput").ap(),
        "out": nc.dram_tensor("out", [T, D], F32, kind="ExternalOutput").ap(),
    }
    _build_tile_kernel(nc, aps)
    nc.compile()
    _CACHE["nc"] = nc
    return nc


def kernel(x, w_qkv, b_qkv, w_proj, b_proj):
    import ml_dtypes
    bf = ml_dtypes.bfloat16

    x = np.ascontiguousarray(np.asarray(x, dtype=np.float32))
    w_qkv = np.asarray(w_qkv, dtype=np.float32)
    b_qkv = np.asarray(b_qkv, dtype=np.float32)
    w_proj = np.asarray(w_proj, dtype=np.float32)
    b_proj = np.asarray(b_proj, dtype=np.float32)

    nc = _get_nc()

    # host-side input prep (dtype cast + layout), shared across cores
    wq = w_qkv[:, :2048].astype(bf)                      # [D, 2048]
    # per pair t: q f-tile t (cols 128t..) then k f-tile t (cols 1024+128t..),
    # each as [128(k-part), 8(k-tile), 128(f)] flattened to [128, 1024]
    wq4 = wq.reshape(8, P, 16, P)                        # [k, p, f, m]
    wqk_prep = np.empty((8, P, 2048), dtype=bf)
    for t in range(8):
        wqk_prep[t, :, 0:1024] = (
            wq4[:, :, t, :].transpose(1, 0, 2).reshape(P, 1024))
        wqk_prep[t, :, 1024:2048] = (
            wq4[:, :, 8 + t, :].transpose(1, 0, 2).reshape(P, 1024))
    vw_prep = np.ascontiguousarray(
        w_qkv[:, 2048:].astype(bf).reshape(8, P, T))     # [k, p, n]
    wp_prep = np.ascontiguousarray(
        w_proj.astype(bf).reshape(8, P, T))              # [c, p, n]
    bcol = np.ascontiguousarray(
        b_qkv[0:2048].reshape(16, P).T.astype(np.float32))
    bv = b_qkv[2048:3072].reshape(1, D).astype(bf)
    bp = b_proj.reshape(1, D).astype(bf)
    r = np.arange(P)
    tri = (r[:, None] <= r[None, :]).astype(bf)
    ones = np.ones((1, P), dtype=bf)

    shared = {
        "wqk": wqk_prep, "vw": vw_prep, "wp": wp_prep,
        "bcol": bcol, "bv": bv, "bp": bp, "tri": tri, "ones": ones,
    }
    in_maps = []
    for b in range(N_CORES):
        xtb = np.ascontiguousarray(
            x[b].T.astype(bf).reshape(8, P, T))          # [k, p, t]
        in_maps.append(dict(shared, xt=xtb))

    res = bass_utils.run_bass_kernel_spmd(
        nc, in_maps, core_ids=list(range(N_CORES)), trace=TRACE
    )
    LAST_RESULT["res"] = res
    return np.stack([res.results[c]["out"] for c in range(N_CORES)]).astype(
        np.float32
    )


# revision 28
# speedup vs baseline: 1.0456x; 1.0136x over previous
"""Causal self-attention Bass kernel for 8x Trainium2 NeuronCores.

Problem: B=8, T=1024, D=1024, H=16 heads (head_dim 64), fp32.
Sharding: data parallel over batch -- each of the 8 cores handles one
batch element with replicated weights; outputs are stacked on the host.

v2 design (vs the 339us baseline): the baseline ran the whole attention
phase with the PE at K=4/8 (HAM-cold, half clock) and serialized 156us
of ACT work after the GEMM phase.  This version:
  * transposes + bf16-casts x on the host (kills 64 PE transposes and
    their DVE evacuations; weights were already host-prepped in the
    baseline),
  * runs the per-head-pair attention chains *interleaved* with the next
    pair's qkv GEMM slice, so the PE always has dense independent matmul
    work while ACT exp catches up (stays HAM-warm),
  * pairs the two heads of an f-tile in concurrent row-tiled QK matmuls
    (tile_position (0,0)/(64,0), K=64 each) -- 2x QK throughput,
  * one exp ACT call per (i-tile, head-pair) over [128,2,512-ws], and
    softmax 1/denom via batched Ln/Exp on [2,512] rows per (pair, j)
    instead of 64 separate [1,512] calls,
  * exact causal windows (width 512-128*(i-4j)).

Per-core dataflow (all matmuls on PE in bf16 with fp32 PSUM accumulate):
  1. v = x @ w_qkv[:, 2048:] + bias (ones-row matmul), stored per
     tk-tile with an interleaved ones column (denominator trick).
  2. Per head pair t: qT/kT f-tiles [128, T] from w_qkv.T @ x.T with
     bias folded into the PSUM->SBUF evacuation (per-partition add).
  3. Per (pair, j-block, i-tile): sT[tk,tq] = kT.T @ qT for both heads
     concurrently (row-tiled), exp on ACT (scale 1/8), triangular mask
     multiply on diagonal tiles, o_aug[65,tq] += v_aug.T @ P.
  4. 1/denom = exp(-ln(d)) batched on [2,512], gpsimd broadcast,
     normalize fused into the PSUM evacuation (DVE tensor_tensor).
  5. y = attT.T @ w_proj + b_proj (ones-row matmul), streamed to DRAM.
"""

import numpy as np
from contextlib import ExitStack

import concourse.bass as bass
import concourse.bacc as bacc
import concourse.tile as tile
import concourse.mybir as mybir
from concourse import bass_utils

F32 = mybir.dt.float32
BF16 = mybir.dt.bfloat16
AF = mybir.ActivationFunctionType
OP = mybir.AluOpType

B, T, D, H, HD = 8, 1024, 1024, 16, 64
P = 128
N_CORES = 8

TRACE = False
_CACHE = {}
LAST_RESULT = {}


def _build_tile_kernel(nc, aps):
    xt, wqk, vw, wp, bcol, bv, bp, tri, ones, out = (
        aps["xt"], aps["wqk"], aps["vw"], aps["wp"], aps["bcol"],
        aps["bv"], aps["bp"], aps["tri"], aps["ones"], aps["out"],
    )

    with tile.TileContext(nc) as tc, ExitStack() as ctx:
        consts = ctx.enter_context(tc.tile_pool(name="consts", bufs=1))
        big = ctx.enter_context(tc.tile_pool(name="big", bufs=1))
        # all 16 qT/kT f-tiles stay live across both j sweeps
        qk_pool = ctx.enter_context(tc.tile_pool(name="qk_pool", bufs=16))
        v_pool = ctx.enter_context(tc.tile_pool(name="v_pool", bufs=8))
        at_pool = ctx.enter_context(tc.tile_pool(name="at_pool", bufs=16))
        p_pool = ctx.enter_context(tc.tile_pool(name="p_pool", bufs=6))
        nrm_pool = ctx.enter_context(tc.tile_pool(name="nrm_pool", bufs=4))
        row_pool = ctx.enter_context(tc.tile_pool(name="row_pool", bufs=4))
        y_pool = ctx.enter_context(tc.tile_pool(name="y_pool", bufs=3))
        # PSUM: ps2 tiles are [128,1024] (2 banks); 3 bufs = 6 banks.
        # o_ps tiles are [65,512] (1 bank); 2 bufs.  Total 8 banks.
        ps2 = ctx.enter_context(tc.tile_pool(name="ps2", bufs=3, space="PSUM"))
        ops = ctx.enter_context(tc.tile_pool(name="ops", bufs=2, space="PSUM"))

        # ---- constants -------------------------------------------------
        tri_sb = consts.tile([P, P], BF16)
        nc.sync.dma_start(out=tri_sb, in_=tri)
        ones_sb = consts.tile([1, P], BF16)
        nc.sync.dma_start(out=ones_sb, in_=ones)
        bcol_sb = consts.tile([P, 16], F32)  # b_qkv[0:2048] as per-partition cols
        nc.sync.dma_start(out=bcol_sb, in_=bcol)
        bv_sb = consts.tile([1, D], BF16)  # v bias as a row
        nc.sync.dma_start(out=bv_sb, in_=bv)
        bp_sb = consts.tile([1, D], BF16)
        nc.sync.dma_start(out=bp_sb, in_=bp)

        # ---- big input tiles (one tile per k-slice so readers only wait
        # for their own slice's DMA), spread across three engine queues in
        # need-order: pair-0 qk weights + x^T first, then v weights, later
        # pairs, w_proj last
        xt_sb = [big.tile([P, T], BF16, name=f"xt{k}") for k in range(8)]
        vw_sb = [big.tile([P, T], BF16, name=f"vw{k}") for k in range(8)]
        wqk_sb = [big.tile([P, 2048], BF16, name=f"wqk{t}") for t in range(8)]
        wp_sb = [big.tile([P, T], BF16, name=f"wp{c}") for c in range(8)]
        qs = [nc.sync, nc.gpsimd, nc.scalar]
        nc.scalar.dma_start(out=wqk_sb[0], in_=wqk[0])
        for k in range(8):
            qs[k % 3].dma_start(out=xt_sb[k], in_=xt[k])
        for k in range(8):
            (nc.sync if k % 2 == 0 else nc.gpsimd).dma_start(
                out=vw_sb[k], in_=vw[k])
        for t in range(1, 8):
            qs[t % 3].dma_start(out=wqk_sb[t], in_=wqk[t])
        for c in range(8):
            (nc.scalar if c % 2 == 0 else nc.sync).dma_start(
                out=wp_sb[c], in_=wp[c])

        # ---- phase V: v in natural layout with interleaved ones col ----
        v_tiles = []
        # per-head stride 66 (132B) keeps every head's va slice 4B-aligned
        for m in range(8):
            vt = v_pool.tile([P, 16 * 66], BF16, name="vt", tag="vt")
            nc.vector.memset(
                vt.rearrange("p (h c) -> p h c", c=66)[:, :, 64:66], 1.0
            )
            v_tiles.append(vt)
        # ---- qk GEMM for one pair: qT then kT f-tiles ------------------
        qk_tiles = {}  # t -> (q_tile, k_tile), each [128, T] bf16

        pending_evacs = []

        def _evac(qk_t, acc, f):
            # evac on ACT (Identity + per-partition bias) -- keeps the
            # DVE stream free of GEMM-gated head-of-line blocking
            nc.scalar.activation(qk_t[:, 0:512], acc[:, 0:512],
                                 AF.Identity, bias=bcol_sb[:, f:f + 1])
            nc.scalar.activation(qk_t[:, 512:1024], acc[:, 512:1024],
                                 AF.Identity, bias=bcol_sb[:, f:f + 1])

        def flush_evacs():
            while pending_evacs:
                _evac(*pending_evacs.pop(0))

        def emit_qk_gemm(t, defer=False):
            tiles = []
            for which in (0, 1):  # 0 = q f-tile, 1 = k f-tile
                f = t if which == 0 else 8 + t
                qk_t = qk_pool.tile([P, T], BF16, name="qk_t", tag="qk")
                acc = ps2.tile([P, T], F32, name="qkacc", tag="ps2")
                for k in range(8):
                    wsl = wqk_sb[t][:, which * 1024 + k * P:
                                    which * 1024 + (k + 1) * P]
                    nc.tensor.matmul(acc[:, 0:512], wsl, xt_sb[k][:, 0:512],
                                     start=(k == 0), stop=(k == 7))
                    nc.tensor.matmul(acc[:, 512:1024], wsl,
                                     xt_sb[k][:, 512:1024],
                                     start=(k == 0), stop=(k == 7))
                if defer and which == 1:
                    # defer the k-tile evac past this block's exps: its ACT
                    # ops otherwise land between GEMM-end and exp-start,
                    # delaying the whole chain ~1.3us (consumer is 2 blocks
                    # away, so the acc can stay live to block end)
                    pending_evacs.append((qk_t, acc, f))
                else:
                    _evac(qk_t, acc, f)
                tiles.append(qk_t)
            qk_tiles[t] = tuple(tiles)

        emit_qk_gemm(0)
        emit_qk_gemm(1)

        # v-GEMM after the first two qk GEMMs: the qk path needs only
        # wqk[0] + x^T (2.5MB) so the PE starts ~15us earlier
        for m in range(8):
            acc = ps2.tile([P, T], F32, name="vacc", tag="ps2")
            for k in range(8):
                xsl = xt_sb[k][:, m * P:(m + 1) * P]
                nc.tensor.matmul(acc[:, 0:512], xsl, vw_sb[k][:, 0:512],
                                 start=(k == 0), stop=False)
                nc.tensor.matmul(acc[:, 512:1024], xsl, vw_sb[k][:, 512:1024],
                                 start=(k == 0), stop=False)
            nc.tensor.matmul(acc[:, 0:512], ones_sb, bv_sb[:, 0:512],
                             start=False, stop=True)
            nc.tensor.matmul(acc[:, 512:1024], ones_sb, bv_sb[:, 512:1024],
                             start=False, stop=True)
            rr = v_tiles[m].rearrange("p (h c) -> p h c", c=66)
            nc.vector.tensor_copy(rr[:, 0:8, 0:64], acc[:, 0:512])
            nc.vector.tensor_copy(rr[:, 8:16, 0:64], acc[:, 512:1024])


        # ---- attention chains, j-outer ---------------------------------
        # denom staging ring: rows 0 and 32 hold the two heads' denoms
        # (32-aligned partition bases); other rows memset once so the
        # batched Ln never reads uninitialized SBUF.
        dstages, rrows, rrbs = [], [], []
        for _ in range(4):
            ds = row_pool.tile([33, 512], F32, name="dstage", tag="dst")
            nc.vector.memset(ds, 1.0)
            dstages.append(ds)
            rrows.append(row_pool.tile([33, 512], F32, name="rrow", tag="rr"))
            rrbs.append(row_pool.tile([33, 512], BF16, name="rrb", tag="rrb"))

        att_tiles = {}  # (t, j) -> [128, 512] bf16 (head 2t rows 0:64, 2t+1 64:128)
        pending = []    # deferred softmax normalizations

        def flush_norm():
            """Emit the deferred 1/denom + scale for the previous block.
            Deferring keeps the norm's ACT ops out of the exp stream's way
            (no head-of-line blocking) and off the o_ps critical path."""
            if not pending:
                return
            at, dstage, rrow, rrb = pending.pop(0)
            nc.scalar.activation(rrow, dstage, AF.Ln)
            nc.scalar.activation(rrb, rrow, AF.Exp, scale=-1.0)
            rtmp = row_pool.tile([1, 512], BF16, name="rtmp", tag="rt")
            nc.vector.tensor_copy(rtmp, rrb[32:33, :])
            # one [128,512] recip tile: head A rows 0:64, head B rows 64:128,
            # so the scale is a single full-width base-0 DVE multiply
            rb = nrm_pool.tile([P, 512], BF16, name="rb", tag="rb")
            nc.gpsimd.partition_broadcast(rb[0:64, :], rrb[0:1, :])
            # gpsimd can't write at partition base 64: bounce via base-0 tile
            rbB = nrm_pool.tile([64, 512], BF16, name="rbB", tag="rbB")
            nc.gpsimd.partition_broadcast(rbB, rtmp)
            nc.vector.tensor_copy(rb[64:128, :], rbB)
            nc.vector.tensor_tensor(at, at, rb, op=OP.mult)

        def emit_chain(t, j):
            q_t, k_t = qk_tiles[t]
            at = at_pool.tile([P, 512], BF16, name="at", tag="at")
            att_tiles[(t, j)] = at
            o_A = ops.tile([65, 512], F32, name="o_A", tag="ops")
            o_B = ops.tile([65, 512], F32, name="o_B", tag="ops")
            ni = 4 * j + 4
            for i in range(ni):
                ws = max(0, P * (i - 4 * j))
                s2 = ps2.tile([P, 2, 512], F32, name="s2", tag="ps2")
                p2 = p_pool.tile([P, 2, 512], BF16, name="p2", tag="p2")
                for hh in range(2):
                    po = hh * 64
                    kT = k_t[po:po + 64, i * P:(i + 1) * P]
                    qT = q_t[po:po + 64, j * 512 + ws:(j + 1) * 512]
                    nc.tensor.matmul(s2[:, hh, ws:], kT, qT,
                                     start=True, stop=True)
                nc.scalar.activation(p2[:, :, ws:], s2[:, :, ws:],
                                     AF.Exp, scale=0.125)
                if i >= 4 * j:
                    # diagonal block: zero the strict upper triangle
                    for hh in range(2):
                        nc.vector.tensor_tensor(
                            p2[:, hh, ws:ws + P], p2[:, hh, ws:ws + P],
                            tri_sb, op=OP.mult,
                        )
                for hh, o_ps in ((0, o_A), (1, o_B)):
                    h = 2 * t + hh
                    va = v_tiles[i].rearrange(
                        "p (h c) -> p h c", c=66)[:, h, 0:65]
                    nc.tensor.matmul(o_ps[:, ws:], va, p2[:, hh, ws:],
                                     start=(i == 0), stop=(i == ni - 1))
            # stage denominators + evacuate unnormalized o (frees o banks
            # quickly); the recip + scale runs deferred via flush_norm()
            dstage = dstages[(2 * t + j) % 4]
            rrow = rrows[(2 * t + j) % 4]
            rrb = rrbs[(2 * t + j) % 4]
            nc.vector.tensor_copy(dstage[0:1, :], o_A[64:65, :])
            nc.vector.tensor_copy(dstage[32:33, :], o_B[64:65, :])
            nc.vector.tensor_copy(at[0:64, :], o_A[0:64, :])
            nc.vector.tensor_copy(at[64:128, :], o_B[0:64, :])
            pending.append((at, dstage, rrow, rrb))

        def emit_proj(mrow):
            j, mi = mrow // 4, mrow % 4
            y_ps = ps2.tile([P, T], F32, name="y_ps", tag="ps2")
            for c in range(8):
                asl = att_tiles[(c, j)][:, mi * P:(mi + 1) * P]
                nc.tensor.matmul(y_ps[:, 0:512], asl, wp_sb[c][:, 0:512],
                                 start=(c == 0), stop=False)
                nc.tensor.matmul(y_ps[:, 512:1024], asl, wp_sb[c][:, 512:1024],
                                 start=(c == 0), stop=False)
            nc.tensor.matmul(y_ps[:, 0:512], ones_sb, bp_sb[:, 0:512],
                             start=False, stop=True)
            nc.tensor.matmul(y_ps[:, 512:1024], ones_sb, bp_sb[:, 512:1024],
                             start=False, stop=True)
            y_sb = y_pool.tile([P, T], F32, name="y_sb", tag="y")
            nc.vector.tensor_copy(y_sb, y_ps)
            nc.sync.dma_start(out=out[mrow * P:(mrow + 1) * P, :], in_=y_sb)

        # sweep j=0 with the qk GEMMs as PE filler (filler first so its
        # PSUM evacuations overlap the chain instead of gating the next one)
        for t in range(8):
            if t < 6:
                emit_qk_gemm(t + 2, defer=True)
            emit_chain(t, 0)
            flush_norm()
            flush_evacs()
        # sweep j=1 with the j=0 projection rows as PE filler
        for t in range(8):
            if t % 2 == 1:
                emit_proj(t // 2)
            emit_chain(t, 1)
            flush_norm()
        # remaining projection rows (j=1)
        flush_norm()
        for mrow in range(4, 8):
            emit_proj(mrow)


def _pin_act_table(arch):
    """Force every ACT func we use into one table so walrus never emits
    mid-kernel ACT_TABLE_LOADs (each is ~1.3us on the ScalarE stream)."""
    import concourse.hw_specs as hw_specs
    tabs = hw_specs.get_activation_tables(arch)
    keep = "natural_log_exp_and_others"
    if keep not in tabs:
        return
    need = tabs[keep] & {AF.Exp, AF.Ln, AF.Copy, AF.Identity}
    for name, fns in tabs.items():
        if name != keep:
            fns -= need


def _get_nc():
    if "nc" in _CACHE:
        return _CACHE["nc"]
    nc = bacc.Bacc("TRN2", target_bir_lowering=False, debug=False,
                   num_devices=N_CORES)
    _pin_act_table(nc.m.arch)
    aps = {
        "xt": nc.dram_tensor("xt", [8, P, T], BF16, kind="ExternalInput").ap(),
        "wqk": nc.dram_tensor("wqk", [8, P, 2048], BF16, kind="ExternalInput").ap(),
        "vw": nc.dram_tensor("vw", [8, P, T], BF16, kind="ExternalInput").ap(),
        "wp": nc.dram_tensor("wp", [8, P, T], BF16, kind="ExternalInput").ap(),
        "bcol": nc.dram_tensor("bcol", [P, 16], F32, kind="ExternalInput").ap(),
        "bv": nc.dram_tensor("bv", [1, D], BF16, kind="ExternalInput").ap(),
        "bp": nc.dram_tensor("bp", [1, D], BF16, kind="ExternalInput").ap(),
        "tri": nc.dram_tensor("tri", [P, P], BF16, kind="ExternalInput").ap(),
        "ones": nc.dram_tensor("ones", [1, P], BF16, kind="ExternalInput").ap(),
        "out": nc.dram_tensor("out", [T, D], F32, kind="ExternalOutput").ap(),
    }
    _build_tile_kernel(nc, aps)
    nc.compile()
    _CACHE["nc"] = nc
    return nc


def kernel(x, w_qkv, b_qkv, w_proj, b_proj):
    import ml_dtypes
    bf = ml_dtypes.bfloat16

    x = np.ascontiguousarray(np.asarray(x, dtype=np.float32))
    w_qkv = np.asarray(w_qkv, dtype=np.float32)
    b_qkv = np.asarray(b_qkv, dtype=np.float32)
    w_proj = np.asarray(w_proj, dtype=np.float32)
    b_proj = np.asarray(b_proj, dtype=np.float32)

    nc = _get_nc()

    # host-side input prep (dtype cast + layout), shared across cores
    wq = w_qkv[:, :2048].astype(bf)                      # [D, 2048]
    # per pair t: q f-tile t (cols 128t..) then k f-tile t (cols 1024+128t..),
    # each as [128(k-part), 8(k-tile), 128(f)] flattened to [128, 1024]
    wq4 = wq.reshape(8, P, 16, P)                        # [k, p, f, m]
    wqk_prep = np.empty((8, P, 2048), dtype=bf)
    for t in range(8):
        wqk_prep[t, :, 0:1024] = (
            wq4[:, :, t, :].transpose(1, 0, 2).reshape(P, 1024))
        wqk_prep[t, :, 1024:2048] = (
            wq4[:, :, 8 + t, :].transpose(1, 0, 2).reshape(P, 1024))
    vw_prep = np.ascontiguousarray(
        w_qkv[:, 2048:].astype(bf).reshape(8, P, T))     # [k, p, n]
    wp_prep = np.ascontiguousarray(
        w_proj.astype(bf).reshape(8, P, T))              # [c, p, n]
    bcol = np.ascontiguousarray(
        b_qkv[0:2048].reshape(16, P).T.astype(np.float32))
    bv = b_qkv[2048:3072].reshape(1, D).astype(bf)
    bp = b_proj.reshape(1, D).astype(bf)
    r = np.arange(P)
    tri = (r[:, None] <= r[None, :]).astype(bf)
    ones = np.ones((1, P), dtype=bf)

    shared = {
        "wqk": wqk_prep, "vw": vw_prep, "wp": wp_prep,
        "bcol": bcol, "bv": bv, "bp": bp, "tri": tri, "ones": ones,
    }
    in_maps = []
    for b in range(N_CORES):
        xtb = np.ascontiguousarray(
            x[b].T.astype(bf).reshape(8, P, T))          # [k, p, t]
        in_maps.append(dict(shared, xt=xtb))

    res = bass_utils.run_bass_kernel_spmd(
        nc, in_maps, core_ids=list(range(N_CORES)), trace=TRACE
    )
    LAST_RESULT["res"] = res
    return np.stack([res.results[c]["out"] for c in range(N_CORES)]).astype(
        np.float32
    )
